# revision 24
# baseline (speedup 1.0000x reference)
"""Trainium2 Bass kernel for nn_CrossAttentionModule (B=4, C=2048, H=W=32).

The module is two independent cross-attention streams per batch element
(RGB queries over index features, and index queries over RGB features).
That yields 8 perfectly independent units = 4 batches x 2 streams; one
unit per NeuronCore, zero collectives.

Algebraic refactor (saves a full [C,C]x[C,N] conv per unit): Q and K are
only ever used inside softmax(Q^T K / sqrt(C)), and V only in V attn^T, so

  S^T = K^T Q = Xkv^T (Wk^T Wq/sqrt(C)) Xq  +  Xkv^T Wk^T bq/sqrt(C)  + col-terms
      = Xkv^T T,   T = M Xq + p   with  M = Wk^T Wq/sqrt(C),  p = Wk^T bq/sqrt(C)
    (M, p precomputed on the host; the bk-dependent terms vary only with the
     query column and cancel exactly in the softmax normalization, so they
     are dropped)
  E  = exp(S^T)      (no max subtraction: |scores| <~ 5, safe in fp32;
                      softmax ratios are mathematically identical)
  colsum = ones[128,128]^T @ E  -- PE partition-reduce that lands the SAME
           sum on every partition: the reciprocal needs no broadcast
  Z  = (Xkv E^T) * (1/colsum)               [C, N]
  O  = Wv Z + bv                            [C, N]  (attn^T columns sum to 1,
                                             so bv is a plain per-row bias)

Device phases (all matmuls bf16, fp32 PSUM accumulate), each a dense
weight- or activation-stationary matmul stream with no transposes:
  T conv (512 mm) -> S^T (256 mm) + exp -> Z (256 mm) -> O conv (512 mm)

Host side: pre-computes M/p (two 2048^3 f32 GEMMs, ~0.3s), pre-tiles all
weights into the exact slab layout the kernel streams (every DMA is
contiguous), casts to bf16, distributes the 8 units across cores, and
reassembles the 4 reference outputs.
"""

import math
from functools import lru_cache

import ml_dtypes
import numpy as np

B, C, HW, N = 4, 2048, 32, 1024
P = 128
CT = C // P           # 16 channel tiles
NT = N // P           # 8 pixel tiles
KHALF = 512           # moving free dim per matmul

_BF16 = ml_dtypes.bfloat16


def _build_program():
    import concourse.bass as bass
    import concourse.mybir as mybir
    import concourse.tile as tile
    from concourse import bacc

    dtb = mybir.dt.bfloat16
    dtf = mybir.dt.float32

    nc = bacc.Bacc("TRN2", target_bir_lowering=False, debug=False)

    xq_d = nc.declare_dram_parameter("xq", [C, N], dtb, isOutput=False)
    xkv_d = nc.declare_dram_parameter("xkv", [C, N], dtb, isOutput=False)
    xkvt_d = nc.declare_dram_parameter("xkvt", [N, C], dtb, isOutput=False)
    wm_d = nc.declare_dram_parameter("wm", [CT, P, CT, P], dtb, isOutput=False)
    wv_d = nc.declare_dram_parameter("wv", [CT, P, CT, P], dtb, isOutput=False)
    pm_d = nc.declare_dram_parameter("pm", [P, CT], dtf, isOutput=False)
    bv_d = nc.declare_dram_parameter("bv", [P, CT], dtf, isOutput=False)
    out_d = nc.declare_dram_parameter("out", [C, N], dtf, isOutput=True)

    with tile.TileContext(nc) as tc:
        with (
            tc.tile_pool(name="const", bufs=1) as const_pool,
            tc.tile_pool(name="big", bufs=1) as big_pool,
            tc.tile_pool(name="wqk", bufs=3) as wqk_pool,
            tc.tile_pool(name="ostage", bufs=3) as ostage_pool,
            # scores/Z PSUM lives OUTSIDE the conv pool's banks: no phase
            # ever waits for another phase's banks to drain
            tc.tile_pool(name="pss", bufs=2, space=bass.MemorySpace.PSUM) as pss,
        ):
            ones128 = const_pool.tile([P, P], dtb)
            nc.gpsimd.memset(ones128[:], 1.0)
            pm_sb = const_pool.tile([P, CT], dtf)
            bv_sb = const_pool.tile([P, CT], dtf)

            t_sb = big_pool.tile([P, CT, N], dtb)     # T[c, n]
            xkv_sb = big_pool.tile([P, CT, N], dtb)   # Xkv[c, m]

            with tc.tile_pool(
                name="psconv", bufs=4, space=bass.MemorySpace.PSUM
            ) as psconv:

                def conv(x_sb, w_dram, b_sb, epilogue, slab0=None, slab1=None,
                         extra_dma=None, interleave_first=False):
                    # epilogue(ot, ps0, ps1) drains the two finished chains.
                    # With interleave_first, the first two output tiles run
                    # as 4 interleaved PSUM chains: each arriving x-tile
                    # feeds 4 matmuls instead of 2, matching the PE rate to
                    # the DMA arrival rate while x streams in at startup.
                    ot = 0
                    if interleave_first:
                        slabs = [slab0, slab1]
                        chains = []
                        for i in range(4):
                            ch = psconv.tile([P, KHALF], dtf, tag="mm",
                                             name=f"cq{i}")
                            chains.append(ch)
                        for kt in range(CT):
                            for i in range(4):
                                nc.tensor.matmul(
                                    chains[i][:], slabs[i // 2][:, kt, :],
                                    x_sb[:, kt,
                                         (i % 2) * KHALF : (i % 2 + 1) * KHALF],
                                    start=(kt == 0), stop=(kt == CT - 1),
                                )
                        epilogue(0, chains[0], chains[1])
                        epilogue(1, chains[2], chains[3])
                        if extra_dma is not None:
                            extra_dma(0)
                            extra_dma(1)
                        ot = 2
                    for ot in range(ot, CT):
                        if extra_dma is not None:
                            extra_dma(ot)
                        if ot == 0 and slab0 is not None:
                            w_slab = slab0
                        else:
                            w_slab = wqk_pool.tile([P, CT, P], dtb, tag="wslab")
                            nc.sync.dma_start(w_slab[:], w_dram[ot])
                        ps0 = psconv.tile([P, KHALF], dtf, tag="mm")
                        ps1 = psconv.tile([P, KHALF], dtf, tag="mm")
                        for kt in range(CT):
                            nc.tensor.matmul(
                                ps0[:], w_slab[:, kt, :], x_sb[:, kt, 0:KHALF],
                                start=(kt == 0), stop=(kt == CT - 1),
                            )
                            nc.tensor.matmul(
                                ps1[:], w_slab[:, kt, :], x_sb[:, kt, KHALF:N],
                                start=(kt == 0), stop=(kt == CT - 1),
                            )
                        epilogue(ot, ps0, ps1)

                # ---- phase 1: T = M Xq + p ----
                with tc.tile_pool(name="xq", bufs=1) as xq_pool:
                    xq_sb = xq_pool.tile([P, CT, N], dtb)

                    # the first matmul needs only slab0's kt=0..1 slice and
                    # xq tile 0 -- put those tiny pieces at the head of the
                    # DMA queue so compute starts right after queue bring-up
                    w_slab0 = wqk_pool.tile([P, CT, P], dtb, tag="wslab")
                    nc.sync.dma_start(w_slab0[:, 0:2, :], wm_d[0][:, 0:2, :])
                    nc.sync.dma_start(xq_sb[:, 0, :], xq_d[0:P, :])
                    nc.sync.dma_start(w_slab0[:, 2:CT, :], wm_d[0][:, 2:CT, :])
                    w_slab1 = wqk_pool.tile([P, CT, P], dtb, tag="wslab")
                    nc.sync.dma_start(w_slab1[:], wm_d[1])
                    nc.sync.dma_start(pm_sb[:], pm_d[:])
                    nc.sync.dma_start(bv_sb[:], bv_d[:])
                    for kt in range(1, CT):
                        nc.sync.dma_start(
                            xq_sb[:, kt, :], xq_d[kt * P : (kt + 1) * P, :]
                        )

                    def t_epilogue(ot, ps0, ps1):
                        nc.vector.tensor_scalar_add(
                            t_sb[:, ot, 0:KHALF], ps0[:], pm_sb[:, ot : ot + 1]
                        )
                        nc.vector.tensor_scalar_add(
                            t_sb[:, ot, KHALF:N], ps1[:], pm_sb[:, ot : ot + 1]
                        )

                    # stagger the xkv loads through the T conv so they don't
                    # compete with xq for HBM bandwidth at kernel start
                    def load_xkv(ot):
                        nc.sync.dma_start(
                            xkv_sb[:, ot, :], xkv_d[ot * P : (ot + 1) * P, :]
                        )

                    conv(xq_sb, wm_d, pm_sb, t_epilogue, slab0=w_slab0,
                         slab1=w_slab1, extra_dma=load_xkv,
                         interleave_first=True)
                # xq's SBUF zone is free; the attention buffers reuse it

                with tc.tile_pool(name="attn", bufs=1) as attn_pool:
                    xkvt_sb = attn_pool.tile([P, NT, C], dtb)  # Xkv^T[m, c]
                    et_sb = attn_pool.tile([P, NT, N], dtb)    # E = exp(S^T)
                    rb_sb = attn_pool.tile([P, N], dtf)        # 1/colsum

                    for mt in range(NT):
                        nc.sync.dma_start(
                            xkvt_sb[:, mt, :], xkvt_d[mt * P : (mt + 1) * P, :]
                        )

                    # colsums accumulate on every partition via ones-matmuls;
                    # two single-bank tiles in the (idle until O conv) conv
                    # pool
                    sums = [
                        psconv.tile([P, KHALF], dtf, tag="mm", name=f"sums{i}")
                        for i in (0, 1)
                    ]

                    def colsum(mt):
                        for nh in range(2):
                            nc.tensor.matmul(
                                sums[nh][:],
                                ones128[:],
                                et_sb[:, mt, nh * KHALF : (nh + 1) * KHALF],
                                start=(mt == 0), stop=(mt == NT - 1),
                            )

                    # ---- phase 2: S^T = Xkv^T T, E = exp(S^T) ----
                    for mt in range(NT):
                        ps = pss.tile([P, N], dtf, tag="s")  # two banks
                        for nh in range(2):
                            for kt in range(CT):
                                nc.tensor.matmul(
                                    ps[:, nh * KHALF : (nh + 1) * KHALF],
                                    xkv_sb[:, kt, mt * P : (mt + 1) * P],
                                    t_sb[:, kt, nh * KHALF : (nh + 1) * KHALF],
                                    start=(kt == 0), stop=(kt == CT - 1),
                                )
                        nc.scalar.activation(
                            et_sb[:, mt, :], ps[:],
                            mybir.ActivationFunctionType.Exp,
                        )
                        # one block late so it never waits on its own exp
                        if mt >= 1:
                            colsum(mt - 1)

                    with tc.tile_pool(name="z", bufs=1) as z_pool:
                        z_sb = z_pool.tile([P, CT, N], dtb)  # Z[c, n]

                        # ---- phase 3: Z = (Xkv E^T) / colsum ----
                        for ct in range(CT):
                            ps = pss.tile([P, N], dtf, tag="s", name=f"z{ct}")
                            for nh in range(2):
                                for mt in range(NT):
                                    nc.tensor.matmul(
                                        ps[:, nh * KHALF : (nh + 1) * KHALF],
                                        xkvt_sb[:, mt, ct * P : (ct + 1) * P],
                                        et_sb[:, mt,
                                              nh * KHALF : (nh + 1) * KHALF],
                                        start=(mt == 0), stop=(mt == NT - 1),
                                    )
                            if ct == 0:
                                # by now exp(mt=7) is long done; the
                                # reciprocal overlaps the next chains
                                colsum(NT - 1)
                                nc.vector.reciprocal_approx_fast(
                                    rb_sb[:, 0:KHALF], sums[0][:]
                                )
                                nc.vector.reciprocal_approx_fast(
                                    rb_sb[:, KHALF:N], sums[1][:]
                                )
                            nc.vector.tensor_mul(
                                z_sb[:, ct, 0:KHALF], ps[:, 0:KHALF],
                                rb_sb[:, 0:KHALF],
                            )
                            nc.vector.tensor_mul(
                                z_sb[:, ct, KHALF:N], ps[:, KHALF:N],
                                rb_sb[:, KHALF:N],
                            )

                        # ---- phase 4: O = Wv Z + bv -> DRAM ----
                        def o_epilogue(ot, ps0, ps1):
                            # per-half stage + DMA so the kernel tail only
                            # exposes half an epilogue before the barrier
                            o_stage = ostage_pool.tile([P, N], dtf, tag="o")
                            for nh, ps in ((0, ps0), (1, ps1)):
                                half = o_stage[:, nh * KHALF : (nh + 1) * KHALF]
                                nc.vector.tensor_scalar_add(
                                    half, ps[:], bv_sb[:, ot : ot + 1]
                                )
                                nc.sync.dma_start(
                                    out_d[ot * P : (ot + 1) * P,
                                          nh * KHALF : (nh + 1) * KHALF],
                                    half,
                                )

                        conv(z_sb, wv_d, bv_sb, o_epilogue)

    nc.compile()
    return nc


@lru_cache(maxsize=1)
def _get_nc():
    return _build_program()


def _tile_w(W_oc):
    # [o, c] weight -> [ot, ci, kt, o] bf16 tiles: lhsT tile (ot, kt) is the
    # contiguous [128, 128] block W.T[kt*128:, ot*128:]
    WT = np.ascontiguousarray(W_oc.T)
    return np.ascontiguousarray(
        WT.reshape(CT, P, CT, P).transpose(2, 1, 0, 3)
    ).astype(_BF16)


def _pack_b(b):
    return np.ascontiguousarray(b.reshape(CT, P).T).astype(np.float32)


def _prep_stream(Wq, bq, Wk, Wv, bv):
    scale = 1.0 / math.sqrt(C)
    M = (Wk.T @ Wq) * scale       # [c_kv, c_q]... as [o=c_kv, c=c_q]
    p = (Wk.T @ bq) * scale
    return dict(
        wm=_tile_w(M), pm=_pack_b(p), wv=_tile_w(Wv), bv=_pack_b(bv)
    )


def _run(inputs, trace=False):
    from concourse.bass_utils import run_bass_kernel_spmd

    F_rgb = np.asarray(inputs["F_rgb"], dtype=np.float32)
    F_ind = np.asarray(inputs["F_indices"], dtype=np.float32)

    g = {k: np.asarray(v, np.float32) for k, v in inputs.items()
         if k.startswith(("W_", "b_"))}
    # stream 0: rgb queries attend over index features
    s0 = _prep_stream(g["W_q_rgb"], g["b_q_rgb"], g["W_k_ind"],
                      g["W_v_ind"], g["b_v_ind"])
    # stream 1: index queries attend over rgb features
    s1 = _prep_stream(g["W_q_ind"], g["b_q_ind"], g["W_k_rgb"],
                      g["W_v_rgb"], g["b_v_rgb"])

    rgb_flat = [np.ascontiguousarray(F_rgb[b].reshape(C, N)).astype(_BF16)
                for b in range(B)]
    ind_flat = [np.ascontiguousarray(F_ind[b].reshape(C, N)).astype(_BF16)
                for b in range(B)]
    rgb_t = [np.ascontiguousarray(F_rgb[b].reshape(C, N).T).astype(_BF16)
             for b in range(B)]
    ind_t = [np.ascontiguousarray(F_ind[b].reshape(C, N).T).astype(_BF16)
             for b in range(B)]

    in_maps = []
    for b in range(B):  # cores 0-3: stream 0 (kv = index features)
        in_maps.append(dict(xq=rgb_flat[b], xkv=ind_flat[b], xkvt=ind_t[b],
                            **s0))
    for b in range(B):  # cores 4-7: stream 1 (kv = rgb features)
        in_maps.append(dict(xq=ind_flat[b], xkv=rgb_flat[b], xkvt=rgb_t[b],
                            **s1))

    nc = _get_nc()
    res = run_bass_kernel_spmd(nc, in_maps, core_ids=list(range(8)), trace=trace)

    O1 = np.stack([res.results[b]["out"].reshape(C, HW, HW) for b in range(B)])
    O2 = np.stack([res.results[4 + b]["out"].reshape(C, HW, HW) for b in range(B)])
    F_final = O1 + O2
    attention_weights = np.stack([O1, O2], axis=1)
    return (F_final, F_rgb, F_ind, attention_weights), res


def kernel(**inputs):
    outs, _ = _run(inputs, trace=False)
    return outs


def kernel_profiled(**inputs):
    outs, res = _run(inputs, trace=True)
    return outs, res


# revision 25
# speedup vs baseline: 1.0089x; 1.0089x over previous
"""Trainium2 Bass kernel for nn_CrossAttentionModule (B=4, C=2048, H=W=32).

The module is two independent cross-attention streams per batch element
(RGB queries over index features, and index queries over RGB features).
That yields 8 perfectly independent units = 4 batches x 2 streams; one
unit per NeuronCore, zero collectives.

Algebraic refactor (saves a full [C,C]x[C,N] conv per unit): Q and K are
only ever used inside softmax(Q^T K / sqrt(C)), and V only in V attn^T, so

  S^T = K^T Q = Xkv^T (Wk^T Wq/sqrt(C)) Xq  +  Xkv^T Wk^T bq/sqrt(C)  + col-terms
      = Xkv^T T,   T = M Xq + p   with  M = Wk^T Wq/sqrt(C),  p = Wk^T bq/sqrt(C)
    (M, p precomputed on the host; the bk-dependent terms vary only with the
     query column and cancel exactly in the softmax normalization, so they
     are dropped)
  E  = exp(S^T)      (no max subtraction: |scores| <~ 5, safe in fp32;
                      softmax ratios are mathematically identical)
  colsum = ones[128,128]^T @ E  -- PE partition-reduce that lands the SAME
           sum on every partition: the reciprocal needs no broadcast
  Z  = (Xkv E^T) * (1/colsum)               [C, N]
  O  = Wv Z + bv                            [C, N]  (attn^T columns sum to 1,
                                             so bv is a plain per-row bias)

Device phases (all matmuls bf16, fp32 PSUM accumulate), each a dense
weight- or activation-stationary matmul stream with no transposes:
  T conv (512 mm) -> S^T (256 mm) + exp -> Z (256 mm) -> O conv (512 mm)

Host side: pre-computes M/p (two 2048^3 f32 GEMMs, ~0.3s), pre-tiles all
weights into the exact slab layout the kernel streams (every DMA is
contiguous), casts to bf16, distributes the 8 units across cores, and
reassembles the 4 reference outputs.
"""

import math
from functools import lru_cache

import ml_dtypes
import numpy as np

B, C, HW, N = 4, 2048, 32, 1024
P = 128
CT = C // P           # 16 channel tiles
NT = N // P           # 8 pixel tiles
KHALF = 512           # moving free dim per matmul

_BF16 = ml_dtypes.bfloat16


def _build_program():
    import concourse.bass as bass
    import concourse.mybir as mybir
    import concourse.tile as tile
    from concourse import bacc

    dtb = mybir.dt.bfloat16
    dtf = mybir.dt.float32

    nc = bacc.Bacc("TRN2", target_bir_lowering=False, debug=False)

    xq_d = nc.declare_dram_parameter("xq", [C, N], dtb, isOutput=False)
    xkv_d = nc.declare_dram_parameter("xkv", [C, N], dtb, isOutput=False)
    xkvt_d = nc.declare_dram_parameter("xkvt", [N, C], dtb, isOutput=False)
    wm_d = nc.declare_dram_parameter("wm", [CT, P, CT, P], dtb, isOutput=False)
    wv_d = nc.declare_dram_parameter("wv", [CT, P, CT, P], dtb, isOutput=False)
    pm_d = nc.declare_dram_parameter("pm", [P, CT], dtf, isOutput=False)
    bv_d = nc.declare_dram_parameter("bv", [P, CT], dtf, isOutput=False)
    out_d = nc.declare_dram_parameter("out", [C, N], dtf, isOutput=True)

    with tile.TileContext(nc) as tc:
        with (
            tc.tile_pool(name="const", bufs=1) as const_pool,
            tc.tile_pool(name="big", bufs=1) as big_pool,
            tc.tile_pool(name="wqk", bufs=3) as wqk_pool,
            tc.tile_pool(name="ostage", bufs=3) as ostage_pool,
            # scores/Z PSUM lives OUTSIDE the conv pool's banks: no phase
            # ever waits for another phase's banks to drain
            tc.tile_pool(name="pss", bufs=2, space=bass.MemorySpace.PSUM) as pss,
        ):
            ones128 = const_pool.tile([P, P], dtb)
            nc.gpsimd.memset(ones128[:], 1.0)
            pm_sb = const_pool.tile([P, CT], dtf)
            bv_sb = const_pool.tile([P, CT], dtf)

            t_sb = big_pool.tile([P, CT, N], dtb)     # T[c, n]
            xkv_sb = big_pool.tile([P, CT, N], dtb)   # Xkv[c, m]

            with tc.tile_pool(
                name="psconv", bufs=4, space=bass.MemorySpace.PSUM
            ) as psconv:

                def conv(x_sb, w_dram, b_sb, epilogue, slab0=None, slab1=None,
                         extra_dma=None, interleave_first=False):
                    # epilogue(ot, ps0, ps1) drains the two finished chains.
                    # With interleave_first, the first two output tiles run
                    # as 4 interleaved PSUM chains: each arriving x-tile
                    # feeds 4 matmuls instead of 2, matching the PE rate to
                    # the DMA arrival rate while x streams in at startup.
                    ot = 0
                    if interleave_first:
                        slabs = [slab0, slab1]
                        chains = []
                        for i in range(4):
                            ch = psconv.tile([P, KHALF], dtf, tag="mm",
                                             name=f"cq{i}")
                            chains.append(ch)
                        for kt in range(CT):
                            for i in range(4):
                                nc.tensor.matmul(
                                    chains[i][:], slabs[i // 2][:, kt, :],
                                    x_sb[:, kt,
                                         (i % 2) * KHALF : (i % 2 + 1) * KHALF],
                                    start=(kt == 0), stop=(kt == CT - 1),
                                )
                        epilogue(0, chains[0], chains[1])
                        epilogue(1, chains[2], chains[3])
                        if extra_dma is not None:
                            extra_dma(0)
                            extra_dma(1)
                        ot = 2
                    for ot in range(ot, CT):
                        if extra_dma is not None:
                            extra_dma(ot)
                        if ot == 0 and slab0 is not None:
                            w_slab = slab0
                        else:
                            w_slab = wqk_pool.tile([P, CT, P], dtb, tag="wslab")
                            nc.sync.dma_start(w_slab[:], w_dram[ot])
                        ps0 = psconv.tile([P, KHALF], dtf, tag="mm")
                        ps1 = psconv.tile([P, KHALF], dtf, tag="mm")
                        for kt in range(CT):
                            nc.tensor.matmul(
                                ps0[:], w_slab[:, kt, :], x_sb[:, kt, 0:KHALF],
                                start=(kt == 0), stop=(kt == CT - 1),
                            )
                            nc.tensor.matmul(
                                ps1[:], w_slab[:, kt, :], x_sb[:, kt, KHALF:N],
                                start=(kt == 0), stop=(kt == CT - 1),
                            )
                        epilogue(ot, ps0, ps1)

                # ---- phase 1: T = M Xq + p ----
                with tc.tile_pool(name="xq", bufs=1) as xq_pool:
                    xq_sb = xq_pool.tile([P, CT, N], dtb)

                    w_slab0 = wqk_pool.tile([P, CT, P], dtb, tag="wslab")
                    nc.sync.dma_start(w_slab0[:], wm_d[0])
                    w_slab1 = wqk_pool.tile([P, CT, P], dtb, tag="wslab")
                    nc.sync.dma_start(w_slab1[:], wm_d[1])
                    nc.sync.dma_start(pm_sb[:], pm_d[:])
                    nc.sync.dma_start(bv_sb[:], bv_d[:])
                    for kt in range(CT):
                        nc.sync.dma_start(
                            xq_sb[:, kt, :], xq_d[kt * P : (kt + 1) * P, :]
                        )

                    def t_epilogue(ot, ps0, ps1):
                        nc.vector.tensor_scalar_add(
                            t_sb[:, ot, 0:KHALF], ps0[:], pm_sb[:, ot : ot + 1]
                        )
                        nc.vector.tensor_scalar_add(
                            t_sb[:, ot, KHALF:N], ps1[:], pm_sb[:, ot : ot + 1]
                        )

                    # stagger the xkv loads through the T conv so they don't
                    # compete with xq for HBM bandwidth at kernel start
                    def load_xkv(ot):
                        nc.sync.dma_start(
                            xkv_sb[:, ot, :], xkv_d[ot * P : (ot + 1) * P, :]
                        )

                    conv(xq_sb, wm_d, pm_sb, t_epilogue, slab0=w_slab0,
                         slab1=w_slab1, extra_dma=load_xkv,
                         interleave_first=True)
                # xq's SBUF zone is free; the attention buffers reuse it

                with tc.tile_pool(name="attn", bufs=1) as attn_pool:
                    xkvt_sb = attn_pool.tile([P, NT, C], dtb)  # Xkv^T[m, c]
                    et_sb = attn_pool.tile([P, NT, N], dtb)    # E = exp(S^T)
                    rb_sb = attn_pool.tile([P, N], dtf)        # 1/colsum

                    for mt in range(NT):
                        nc.sync.dma_start(
                            xkvt_sb[:, mt, :], xkvt_d[mt * P : (mt + 1) * P, :]
                        )

                    # colsums accumulate on every partition via ones-matmuls;
                    # two single-bank tiles in the (idle until O conv) conv
                    # pool
                    sums = [
                        psconv.tile([P, KHALF], dtf, tag="mm", name=f"sums{i}")
                        for i in (0, 1)
                    ]

                    def colsum(mt):
                        for nh in range(2):
                            nc.tensor.matmul(
                                sums[nh][:],
                                ones128[:],
                                et_sb[:, mt, nh * KHALF : (nh + 1) * KHALF],
                                start=(mt == 0), stop=(mt == NT - 1),
                            )

                    # ---- phase 2: S^T = Xkv^T T, E = exp(S^T) ----
                    for mt in range(NT):
                        ps = pss.tile([P, N], dtf, tag="s")  # two banks
                        for nh in range(2):
                            for kt in range(CT):
                                nc.tensor.matmul(
                                    ps[:, nh * KHALF : (nh + 1) * KHALF],
                                    xkv_sb[:, kt, mt * P : (mt + 1) * P],
                                    t_sb[:, kt, nh * KHALF : (nh + 1) * KHALF],
                                    start=(kt == 0), stop=(kt == CT - 1),
                                )
                        nc.scalar.activation(
                            et_sb[:, mt, :], ps[:],
                            mybir.ActivationFunctionType.Exp,
                        )
                        # one block late so it never waits on its own exp
                        if mt >= 1:
                            colsum(mt - 1)

                    with tc.tile_pool(name="z", bufs=1) as z_pool:
                        z_sb = z_pool.tile([P, CT, N], dtb)  # Z[c, n]

                        # ---- phase 3: Z = (Xkv E^T) / colsum ----
                        for ct in range(CT):
                            ps = pss.tile([P, N], dtf, tag="s", name=f"z{ct}")
                            for nh in range(2):
                                for mt in range(NT):
                                    nc.tensor.matmul(
                                        ps[:, nh * KHALF : (nh + 1) * KHALF],
                                        xkvt_sb[:, mt, ct * P : (ct + 1) * P],
                                        et_sb[:, mt,
                                              nh * KHALF : (nh + 1) * KHALF],
                                        start=(mt == 0), stop=(mt == NT - 1),
                                    )
                            if ct == 0:
                                # by now exp(mt=7) is long done; the
                                # reciprocal overlaps the next chains
                                colsum(NT - 1)
                                nc.vector.reciprocal_approx_fast(
                                    rb_sb[:, 0:KHALF], sums[0][:]
                                )
                                nc.vector.reciprocal_approx_fast(
                                    rb_sb[:, KHALF:N], sums[1][:]
                                )
                            nc.vector.tensor_mul(
                                z_sb[:, ct, 0:KHALF], ps[:, 0:KHALF],
                                rb_sb[:, 0:KHALF],
                            )
                            nc.vector.tensor_mul(
                                z_sb[:, ct, KHALF:N], ps[:, KHALF:N],
                                rb_sb[:, KHALF:N],
                            )

                        # ---- phase 4: O = Wv Z + bv -> DRAM ----
                        def o_epilogue(ot, ps0, ps1):
                            # per-half stage + DMA so the kernel tail only
                            # exposes half an epilogue before the barrier
                            o_stage = ostage_pool.tile([P, N], dtf, tag="o")
                            for nh, ps in ((0, ps0), (1, ps1)):
                                half = o_stage[:, nh * KHALF : (nh + 1) * KHALF]
                                nc.vector.tensor_scalar_add(
                                    half, ps[:], bv_sb[:, ot : ot + 1]
                                )
                                nc.sync.dma_start(
                                    out_d[ot * P : (ot + 1) * P,
                                          nh * KHALF : (nh + 1) * KHALF],
                                    half,
                                )

                        conv(z_sb, wv_d, bv_sb, o_epilogue)

    nc.compile()
    return nc


@lru_cache(maxsize=1)
def _get_nc():
    return _build_program()


def _tile_w(W_oc):
    # [o, c] weight -> [ot, ci, kt, o] bf16 tiles: lhsT tile (ot, kt) is the
    # contiguous [128, 128] block W.T[kt*128:, ot*128:]
    WT = np.ascontiguousarray(W_oc.T)
    return np.ascontiguousarray(
        WT.reshape(CT, P, CT, P).transpose(2, 1, 0, 3)
    ).astype(_BF16)


def _pack_b(b):
    return np.ascontiguousarray(b.reshape(CT, P).T).astype(np.float32)


def _prep_stream(Wq, bq, Wk, Wv, bv):
    scale = 1.0 / math.sqrt(C)
    M = (Wk.T @ Wq) * scale       # [c_kv, c_q]... as [o=c_kv, c=c_q]
    p = (Wk.T @ bq) * scale
    return dict(
        wm=_tile_w(M), pm=_pack_b(p), wv=_tile_w(Wv), bv=_pack_b(bv)
    )


def _run(inputs, trace=False):
    from concourse.bass_utils import run_bass_kernel_spmd

    F_rgb = np.asarray(inputs["F_rgb"], dtype=np.float32)
    F_ind = np.asarray(inputs["F_indices"], dtype=np.float32)

    g = {k: np.asarray(v, np.float32) for k, v in inputs.items()
         if k.startswith(("W_", "b_"))}
    # stream 0: rgb queries attend over index features
    s0 = _prep_stream(g["W_q_rgb"], g["b_q_rgb"], g["W_k_ind"],
                      g["W_v_ind"], g["b_v_ind"])
    # stream 1: index queries attend over rgb features
    s1 = _prep_stream(g["W_q_ind"], g["b_q_ind"], g["W_k_rgb"],
                      g["W_v_rgb"], g["b_v_rgb"])

    rgb_flat = [np.ascontiguousarray(F_rgb[b].reshape(C, N)).astype(_BF16)
                for b in range(B)]
    ind_flat = [np.ascontiguousarray(F_ind[b].reshape(C, N)).astype(_BF16)
                for b in range(B)]
    rgb_t = [np.ascontiguousarray(F_rgb[b].reshape(C, N).T).astype(_BF16)
             for b in range(B)]
    ind_t = [np.ascontiguousarray(F_ind[b].reshape(C, N).T).astype(_BF16)
             for b in range(B)]

    in_maps = []
    for b in range(B):  # cores 0-3: stream 0 (kv = index features)
        in_maps.append(dict(xq=rgb_flat[b], xkv=ind_flat[b], xkvt=ind_t[b],
                            **s0))
    for b in range(B):  # cores 4-7: stream 1 (kv = rgb features)
        in_maps.append(dict(xq=ind_flat[b], xkv=rgb_flat[b], xkvt=rgb_t[b],
                            **s1))

    nc = _get_nc()
    res = run_bass_kernel_spmd(nc, in_maps, core_ids=list(range(8)), trace=trace)

    O1 = np.stack([res.results[b]["out"].reshape(C, HW, HW) for b in range(B)])
    O2 = np.stack([res.results[4 + b]["out"].reshape(C, HW, HW) for b in range(B)])
    F_final = O1 + O2
    attention_weights = np.stack([O1, O2], axis=1)
    return (F_final, F_rgb, F_ind, attention_weights), res


def kernel(**inputs):
    outs, _ = _run(inputs, trace=False)
    return outs


def kernel_profiled(**inputs):
    outs, res = _run(inputs, trace=True)
    return outs, res
